# revision 19
# baseline (speedup 1.0000x reference)
"""MoE (Gemma-style 8-expert top-2) Trainium2 kernel, v2.

Strategy (intermediate-dim sliced expert parallelism over 8 NeuronCores):
  - Every core processes ALL token->expert assignments, but only a 1/8
    slice (512 cols) of the intermediate dimension I=4096 of every expert.
    Per-core work is therefore sum(C_e)/8 token-equivalents regardless of
    how unbalanced the routing is (vs max(C_e) for one-expert-per-core),
    while per-core weight traffic is identical (1/8 of every expert's
    Wg/Wu/Wd = one full expert's worth).
  - Host: dedup (token,expert) pairs, gather+transpose x into a single
    xT buffer [H, A] (per-expert segments, zero-padded to multiples of 8),
    prepack per-core weight slices so every device DMA is 2D-contiguous.
  - Device (per core r), for each expert block b (descending token count):
        gateT[i,c] = sum_h Wg[h, r*512+i] * xT[h,c]    i in [0,512)
        upT   likewise                                  (fp16 in, fp32 acc)
        hT    = gelu_tanh(gateT) * upT                  [512, C_b] fp16
        ypartT[h,c] = sum_i Wd[r*512+i, h] * hT[i,c]    [H, C_b] -> fp16 out
    Weights stream on the Sync HWDGE ring; x in / y out stream on the
    Scalar (Activation) HWDGE ring so big x transfers never head-of-line
    block the weight stream.  A short 5-matmul warmup trips the PE HAM
    clock-gate while the first DMAs land.
  - Host: sum the 8 cores' fp16 partial yT (f32 accumulate), then
    combine: out[t] += route[t,e] * ysum[:, pos].T, route identical to
    the reference's scatter-add.
"""

import numpy as np

import concourse.bass as bass
import concourse.mybir as mybir
import concourse.tile as tile
from concourse import bacc


def _install_ntff_hook_shim():
    """The agent image's `antenv` lacks `axon_hooks`, which bass_utils
    imports unconditionally when tracing under axon.  Provide the module
    and register the ctypes-based NTFF profile hook so BASS_TRACE=1 yields
    real HW profiles.  Degrades silently if anything is missing."""
    import sys
    import types

    try:
        import antenv

        try:
            from antenv import axon_hooks  # noqa: F401

            return
        except ImportError:
            pass
        mod = types.ModuleType("antenv.axon_hooks")
        mod._hook = None
        mod.set_axon_ntff_profile_hook = lambda h: setattr(mod, "_hook", h)
        mod.get_axon_ntff_profile_hook = lambda: mod._hook
        sys.modules["antenv.axon_hooks"] = mod
        antenv.axon_hooks = mod
        import os

        so_path = "/opt/axon/libaxon_pjrt.so"
        if os.path.exists(so_path):
            from trn_agent_boot.trn_boot import _ntff_profile_via_ctypes

            mod._hook = _ntff_profile_via_ctypes(so_path)
    except Exception:
        pass


_install_ntff_hook_shim()

from concourse.bass_utils import run_bass_kernel_spmd

H = 2048
I = 4096
E = 8
NCORES = 8
P = 128
KH = H // P  # 16 contraction chunks for gate/up
SI = I // NCORES  # 512-wide I slice per core
MI = SI // P  # 4 output i-tiles per expert block
KD = SI // P  # 4 contraction chunks for down
MH = H // P  # 16 output tiles of H
CMAX = 504  # max tokens per block (one PSUM fp32 bank)
F32 = mybir.dt.float32
F16 = mybir.dt.float16

# Results of the last device run (for test harnesses to inspect profiling).
LAST_RESULTS = None

_PROGRAM_CACHE: dict[tuple, "bass.Bass"] = {}


def _build_program(counts: tuple[int, ...]) -> "bass.Bass":
    """Bass program for one core: for each block b, expert-MLP over an
    SI-wide I-slice on counts[b] tokens (transposed layout)."""
    NB = len(counts)
    assert all(8 <= n <= CMAX and n % 4 == 0 for n in counts)
    offs = [0]
    for n in counts:
        offs.append(offs[-1] + n)
    A = offs[-1]

    nc = bacc.Bacc("TRN2", target_bir_lowering=False)

    xT = nc.dram_tensor("xT", [H, A], F16, kind="ExternalInput")
    Wg = nc.dram_tensor("Wg", [NB, MI, P, KH * P], F16, kind="ExternalInput")
    Wu = nc.dram_tensor("Wu", [NB, MI, P, KH * P], F16, kind="ExternalInput")
    Wd = nc.dram_tensor("Wd", [NB, P, MH * KD * P], F16, kind="ExternalInput")
    yT = nc.dram_tensor("yT", [H, A], F16, kind="ExternalOutput")

    xT_r = xT.rearrange("(k p) a -> p k a", p=P)  # [128, 16, A]
    yT_r = yT.rearrange("(m p) a -> p m a", p=P)  # [128, 16, A]
    Wg_a, Wu_a, Wd_a = Wg.ap(), Wu.ap(), Wd.ap()

    gelu = mybir.ActivationFunctionType.Gelu_apprx_tanh

    with tile.TileContext(nc) as tc:
        with (
            tc.tile_pool(name="sb", bufs=1) as sb,
            tc.tile_pool(name="ps", bufs=1, space="PSUM") as ps,
        ):
            # --- PE warmup: a few dummy matmuls trip the HAM clock-gate to
            # 2.4 GHz while the first DMAs land.
            wz = sb.tile([P, P], F16, tag="wz", bufs=1)
            xz = sb.tile([P, CMAX], F16, tag="xz", bufs=1)
            nc.vector.memset(wz, 0.0)
            nc.vector.memset(xz, 0.0)
            pw = ps.tile([P, CMAX], F32, tag="pw", bufs=1)
            # two-stage warmup: big matmuls trip the HAM activity window,
            # then tiny ones keep PE busy through the dep-arrival jitter
            # with minimal drain latency once real work is ready
            for _ in range(10):
                nc.tensor.matmul(pw, wz, xz, start=True, stop=True)
            for _ in range(30):
                nc.tensor.matmul(pw[:, :56], wz, xz[:, :56], start=True, stop=True)

            def issue_x(b, xtile):
                n = counts[b]
                o = offs[b]
                if b == 0:
                    # block 0's first x chunks ride the Scalar ring in
                    # consumption-order pieces; the tail (8:16) rides the
                    # Sync ring after wg0/wu0 (emitted in the gate loop).
                    for s0, s1 in [(0, 3), (3, 8)]:
                        nc.scalar.dma_start(
                            out=xtile[:, s0:s1, :n], in_=xT_r[:, s0:s1, o : o + n]
                        )
                else:
                    nc.scalar.dma_start(
                        out=xtile[:, 0:8, :n], in_=xT_r[:, 0:8, o : o + n]
                    )
                    nc.scalar.dma_start(
                        out=xtile[:, 8:KH, :n], in_=xT_r[:, 8:KH, o : o + n]
                    )

            xt = sb.tile([P, KH, CMAX], F16, tag="x", bufs=2, name="x0")
            issue_x(0, xt)

            for b in range(NB):
                n = counts[b]
                o = offs[b]
                # prefetch next block's x on the Scalar ring
                if b + 1 < NB:
                    xt_next = sb.tile(
                        [P, KH, CMAX], F16, tag="x", bufs=2, name=f"x{b + 1}"
                    )
                    issue_x(b + 1, xt_next)
                else:
                    xt_next = None

                hb = sb.tile([P, KD, CMAX], F16, tag="h", bufs=2, name=f"h{b}")
                yb = sb.tile([P, MH, CMAX], F16, tag="y", bufs=2, name=f"y{b}")

                # ---- Phase 1: gate/up -> h, one i-tile (128 rows) at a time
                for m in range(MI):
                    wgt = sb.tile([P, KH * P], F16, tag="wg", bufs=6, name=f"wg{b}_{m}")
                    nc.sync.dma_start(out=wgt, in_=Wg_a[b, m])
                    wut = sb.tile([P, KH * P], F16, tag="wu", bufs=6, name=f"wu{b}_{m}")
                    nc.sync.dma_start(out=wut, in_=Wu_a[b, m])
                    if b == 0 and m == 0:
                        # tail of block-0 x after wg0/wu0 on the Sync ring
                        # (all x emitted before any matmul that reads it —
                        # dep tracking is program-order)
                        nc.sync.dma_start(
                            out=xt[:, 8:12, :n], in_=xT_r[:, 8:12, o : o + n]
                        )
                        nc.sync.dma_start(
                            out=xt[:, 12:KH, :n], in_=xT_r[:, 12:KH, o : o + n]
                        )
                    wg_r = wgt.rearrange("p (k i) -> p k i", i=P)
                    wu_r = wut.rearrange("p (k i) -> p k i", i=P)

                    pg = ps.tile([P, n], F32, tag="g", bufs=2, name=f"pg{b}_{m}")
                    pu = ps.tile([P, n], F32, tag="u", bufs=2, name=f"pu{b}_{m}")
                    if b == 0 and m == 0:
                        # startup: interleave gate/up per k-chunk so x is
                        # consumed at half the rate while it streams in —
                        # keeps the PE busy right after warmup (no HAM
                        # re-throttle) with only wg0+wu0+chunk0 resident
                        for k in range(KH):
                            nc.tensor.matmul(
                                pg,
                                wg_r[:, k, :],
                                xt[:, k, :n],
                                start=(k == 0),
                                stop=(k == KH - 1),
                            )
                            nc.tensor.matmul(
                                pu,
                                wu_r[:, k, :],
                                xt[:, k, :n],
                                start=(k == 0),
                                stop=(k == KH - 1),
                            )
                    else:
                        for k in range(KH):
                            nc.tensor.matmul(
                                pg,
                                wg_r[:, k, :],
                                xt[:, k, :n],
                                start=(k == 0),
                                stop=(k == KH - 1),
                            )
                        for k in range(KH):
                            nc.tensor.matmul(
                                pu,
                                wu_r[:, k, :],
                                xt[:, k, :n],
                                start=(k == 0),
                                stop=(k == KH - 1),
                            )
                    tg = sb.tile([P, n], F32, tag="tg", bufs=2, name=f"tg{b}_{m}")
                    nc.scalar.activation(tg, pg, gelu)
                    nc.vector.tensor_mul(hb[:, m, :n], tg, pu)

                # down-proj weights for this block (2 halves, Sync ring)
                wdt = sb.tile([P, MH, KD, P], F16, tag="wd", bufs=2, name=f"wd{b}")
                half = (MH // 2) * KD * P
                wdt_f = wdt.rearrange("p m k i -> p (m k i)")
                nc.sync.dma_start(out=wdt_f[:, :half], in_=Wd_a[b, :, :half])
                nc.sync.dma_start(out=wdt_f[:, half:], in_=Wd_a[b, :, half:])

                # ---- Phase 2: down -> y partial, one H-tile at a time
                last_block = b == NB - 1
                for mh in range(MH):
                    if last_block and mh == MH - 1:
                        # final tile: two half-width groups so the first
                        # half's copy+DMA hides under the second's matmuls
                        nh = ((n // 2) + 3) // 4 * 4
                        for c0, c1 in ((0, nh), (nh, n)):
                            pd = ps.tile(
                                [P, c1 - c0], F32, tag="d", bufs=3, name=f"pd{b}_{mh}_{c0}"
                            )
                            for k in range(KD):
                                nc.tensor.matmul(
                                    pd,
                                    wdt[:, mh, k, :],
                                    hb[:, k, c0:c1],
                                    start=(k == 0),
                                    stop=(k == KD - 1),
                                )
                            nc.vector.tensor_copy(yb[:, mh, c0:c1], pd)
                            nc.scalar.dma_start(
                                out=yT_r[:, mh : mh + 1, o + c0 : o + c1],
                                in_=yb[:, mh : mh + 1, c0:c1],
                            )
                        continue
                    pd = ps.tile([P, n], F32, tag="d", bufs=3, name=f"pd{b}_{mh}")
                    for k in range(KD):
                        nc.tensor.matmul(
                            pd,
                            wdt[:, mh, k, :],
                            hb[:, k, :n],
                            start=(k == 0),
                            stop=(k == KD - 1),
                        )
                    nc.vector.tensor_copy(yb[:, mh, :n], pd)
                    # y partials drain on the otherwise-idle gpsimd SWDGE
                    # ring so they never congest the scalar queue (which
                    # must stay responsive for gelu ACTIVATEs) nor delay
                    # the weight stream on sync
                    if last_block and mh >= 13:
                        # tail tiles ride the (idle, low-latency) Scalar
                        # HWDGE ring; a gpsimd DMA here would leave a
                        # multi-us SWDGE drain in the teardown
                        s0 = 12 if mh == 13 else mh
                        nc.scalar.dma_start(
                            out=yT_r[:, s0 : mh + 1, o : o + n],
                            in_=yb[:, s0 : mh + 1, :n],
                        )
                    elif mh % 4 == 3 and not (last_block and mh == MH - 1):
                        nc.gpsimd.dma_start(
                            out=yT_r[:, mh - 3 : mh + 1, o : o + n],
                            in_=yb[:, mh - 3 : mh + 1, :n],
                        )
                xt = xt_next

    nc.compile()
    return nc


def _get_program(counts: tuple[int, ...]) -> "bass.Bass":
    if counts not in _PROGRAM_CACHE:
        _PROGRAM_CACHE[counts] = _build_program(counts)
    return _PROGRAM_CACHE[counts]


def _prep_w_gu_slice(w16, r):
    """[H, I] fp16 -> [MI, P, KH*P] for I-slice r (per-tile contiguous)."""
    s = w16[:, r * SI : (r + 1) * SI]  # [2048, 512]
    return np.ascontiguousarray(
        s.reshape(KH, P, MI, P).transpose(2, 1, 0, 3)
    ).reshape(MI, P, KH * P)


def _prep_w_d_slice(w16, r):
    """[I, H] fp16 -> [P, MH*KD*P] for I-slice r: [p_i, (mh, k, p_h)]."""
    s = w16[r * SI : (r + 1) * SI, :]  # [512, 2048]
    return np.ascontiguousarray(
        s.reshape(KD, P, MH, P).transpose(1, 2, 0, 3)
    ).reshape(P, MH * KD * P)


def kernel(x, selected_experts, routing_weights, Wg, Wu, Wd):
    global LAST_RESULTS
    x = np.asarray(x, dtype=np.float32)
    se = np.asarray(selected_experts).astype(np.int64)
    rw = np.asarray(routing_weights).astype(np.float32)
    Wg = np.asarray(Wg, dtype=np.float32)
    Wu = np.asarray(Wu, dtype=np.float32)
    Wd = np.asarray(Wd, dtype=np.float32)

    T, K = se.shape
    assert x.shape == (T, H) and Wg.shape == (E, H, I) and Wd.shape == (E, I, H)

    # Dense route matrix, identical to the reference's scatter-add (merges
    # duplicate expert picks within a token by summing their weights).
    flat_t = np.repeat(np.arange(T), K)
    flat_e = se.ravel()
    route = np.zeros((T, E), np.float32)
    np.add.at(route, (flat_t, flat_e), rw.ravel())
    present = np.zeros((T, E), bool)
    present[flat_t, flat_e] = True

    # Blocks: one per expert (split if > CMAX), descending size so the
    # device tail lands on the smallest block.
    blocks = []  # (expert, token_index_array)
    for e in range(E):
        ix = np.nonzero(present[:, e])[0]
        for s in range(0, len(ix), CMAX):
            chunk = ix[s : s + CMAX]
            if len(chunk):
                blocks.append((e, chunk))
    blocks.sort(key=lambda be: -len(be[1]))
    if len(blocks) > 1:
        # smallest block first (lightest startup x transfer), rest descending
        blocks = blocks[-1:] + blocks[:-1]
    counts = tuple(max(8, -(-len(ix) // 4) * 4) for _, ix in blocks)
    offs = np.concatenate([[0], np.cumsum(counts)])
    A = int(offs[-1])

    nc = _get_program(counts)

    xT_all = np.zeros((H, A), np.float16)
    for b, (_, ix) in enumerate(blocks):
        xT_all[:, offs[b] : offs[b] + len(ix)] = x[ix].T.astype(np.float16)

    Wg16 = Wg.astype(np.float16)
    Wu16 = Wu.astype(np.float16)
    Wd16 = Wd.astype(np.float16)

    in_maps = []
    for r in range(NCORES):
        wg_r = np.stack([_prep_w_gu_slice(Wg16[e], r) for e, _ in blocks])
        wu_r = np.stack([_prep_w_gu_slice(Wu16[e], r) for e, _ in blocks])
        wd_r = np.stack([_prep_w_d_slice(Wd16[e], r) for e, _ in blocks])
        in_maps.append({"xT": xT_all, "Wg": wg_r, "Wu": wu_r, "Wd": wd_r})

    res = run_bass_kernel_spmd(nc, in_maps, core_ids=list(range(NCORES)))
    LAST_RESULTS = res

    ysum = np.zeros((H, A), np.float32)
    for r in range(NCORES):
        ysum += res.results[r]["yT"].astype(np.float32)

    out = np.zeros((T, H), np.float32)
    for b, (e, ix) in enumerate(blocks):
        seg = ysum[:, offs[b] : offs[b] + len(ix)]
        out[ix] += route[ix, e][:, None] * seg.T
    return out


# revision 21
# speedup vs baseline: 1.0105x; 1.0105x over previous
"""MoE (Gemma-style 8-expert top-2) Trainium2 kernel, v2.

Strategy (intermediate-dim sliced expert parallelism over 8 NeuronCores):
  - Every core processes ALL token->expert assignments, but only a 1/8
    slice (512 cols) of the intermediate dimension I=4096 of every expert.
    Per-core work is therefore sum(C_e)/8 token-equivalents regardless of
    how unbalanced the routing is (vs max(C_e) for one-expert-per-core),
    while per-core weight traffic is identical (1/8 of every expert's
    Wg/Wu/Wd = one full expert's worth).
  - Host: dedup (token,expert) pairs, gather+transpose x into a single
    xT buffer [H, A] (per-expert segments, zero-padded to multiples of 8),
    prepack per-core weight slices so every device DMA is 2D-contiguous.
  - Device (per core r), for each expert block b (descending token count):
        gateT[i,c] = sum_h Wg[h, r*512+i] * xT[h,c]    i in [0,512)
        upT   likewise                                  (fp16 in, fp32 acc)
        hT    = gelu_tanh(gateT) * upT                  [512, C_b] fp16
        ypartT[h,c] = sum_i Wd[r*512+i, h] * hT[i,c]    [H, C_b] -> fp16 out
    Weights stream on the Sync HWDGE ring; x in / y out stream on the
    Scalar (Activation) HWDGE ring so big x transfers never head-of-line
    block the weight stream.  A short 5-matmul warmup trips the PE HAM
    clock-gate while the first DMAs land.
  - Host: sum the 8 cores' fp16 partial yT (f32 accumulate), then
    combine: out[t] += route[t,e] * ysum[:, pos].T, route identical to
    the reference's scatter-add.
"""

import numpy as np

import concourse.bass as bass
import concourse.mybir as mybir
import concourse.tile as tile
from concourse import bacc


def _install_ntff_hook_shim():
    """The agent image's `antenv` lacks `axon_hooks`, which bass_utils
    imports unconditionally when tracing under axon.  Provide the module
    and register the ctypes-based NTFF profile hook so BASS_TRACE=1 yields
    real HW profiles.  Degrades silently if anything is missing."""
    import sys
    import types

    try:
        import antenv

        try:
            from antenv import axon_hooks  # noqa: F401

            return
        except ImportError:
            pass
        mod = types.ModuleType("antenv.axon_hooks")
        mod._hook = None
        mod.set_axon_ntff_profile_hook = lambda h: setattr(mod, "_hook", h)
        mod.get_axon_ntff_profile_hook = lambda: mod._hook
        sys.modules["antenv.axon_hooks"] = mod
        antenv.axon_hooks = mod
        import os

        so_path = "/opt/axon/libaxon_pjrt.so"
        if os.path.exists(so_path):
            from trn_agent_boot.trn_boot import _ntff_profile_via_ctypes

            mod._hook = _ntff_profile_via_ctypes(so_path)
    except Exception:
        pass


_install_ntff_hook_shim()

from concourse.bass_utils import run_bass_kernel_spmd

H = 2048
I = 4096
E = 8
NCORES = 8
P = 128
KH = H // P  # 16 contraction chunks for gate/up
SI = I // NCORES  # 512-wide I slice per core
MI = SI // P  # 4 output i-tiles per expert block
KD = SI // P  # 4 contraction chunks for down
MH = H // P  # 16 output tiles of H
CMAX = 504  # max tokens per block (one PSUM fp32 bank)
F32 = mybir.dt.float32
F16 = mybir.dt.float16

# Results of the last device run (for test harnesses to inspect profiling).
LAST_RESULTS = None

_PROGRAM_CACHE: dict[tuple, "bass.Bass"] = {}


def _build_program(counts: tuple[int, ...]) -> "bass.Bass":
    """Bass program for one core: for each block b, expert-MLP over an
    SI-wide I-slice on counts[b] tokens (transposed layout)."""
    NB = len(counts)
    assert all(8 <= n <= CMAX and n % 4 == 0 for n in counts)
    offs = [0]
    for n in counts:
        offs.append(offs[-1] + n)
    A = offs[-1]

    nc = bacc.Bacc("TRN2", target_bir_lowering=False)

    xT = nc.dram_tensor("xT", [H, A], F16, kind="ExternalInput")
    Wg = nc.dram_tensor("Wg", [NB, MI, P, KH * P], F16, kind="ExternalInput")
    Wu = nc.dram_tensor("Wu", [NB, MI, P, KH * P], F16, kind="ExternalInput")
    Wd = nc.dram_tensor("Wd", [NB, P, MH * KD * P], F16, kind="ExternalInput")
    yT = nc.dram_tensor("yT", [H, A], F16, kind="ExternalOutput")

    xT_r = xT.rearrange("(k p) a -> p k a", p=P)  # [128, 16, A]
    yT_r = yT.rearrange("(m p) a -> p m a", p=P)  # [128, 16, A]
    Wg_a, Wu_a, Wd_a = Wg.ap(), Wu.ap(), Wd.ap()

    gelu = mybir.ActivationFunctionType.Gelu_apprx_tanh

    with tile.TileContext(nc) as tc:
        with (
            tc.tile_pool(name="sb", bufs=1) as sb,
            tc.tile_pool(name="ps", bufs=1, space="PSUM") as ps,
        ):
            # --- PE warmup: a few dummy matmuls trip the HAM clock-gate to
            # 2.4 GHz while the first DMAs land.
            wz = sb.tile([P, P], F16, tag="wz", bufs=1)
            xz = sb.tile([P, CMAX], F16, tag="xz", bufs=1)
            nc.vector.memset(wz, 0.0)
            nc.vector.memset(xz, 0.0)
            pw = ps.tile([P, CMAX], F32, tag="pw", bufs=1)
            # two-stage warmup: big matmuls trip the HAM activity window,
            # then tiny ones keep PE busy through the dep-arrival jitter
            # with minimal drain latency once real work is ready
            for _ in range(10):
                nc.tensor.matmul(pw, wz, xz, start=True, stop=True)
            for _ in range(55):
                nc.tensor.matmul(pw[:, :56], wz, xz[:, :56], start=True, stop=True)

            def issue_x(b, xtile):
                n = counts[b]
                o = offs[b]
                if b == 0:
                    # block 0's first x chunks ride the Scalar ring in
                    # consumption-order pieces; the tail (8:16) rides the
                    # Sync ring after wg0/wu0 (emitted in the gate loop).
                    for s0, s1 in [(0, 3), (3, 8)]:
                        nc.scalar.dma_start(
                            out=xtile[:, s0:s1, :n], in_=xT_r[:, s0:s1, o : o + n]
                        )
                else:
                    nc.scalar.dma_start(
                        out=xtile[:, 0:8, :n], in_=xT_r[:, 0:8, o : o + n]
                    )
                    nc.scalar.dma_start(
                        out=xtile[:, 8:KH, :n], in_=xT_r[:, 8:KH, o : o + n]
                    )

            xt = sb.tile([P, KH, CMAX], F16, tag="x", bufs=2, name="x0")
            issue_x(0, xt)

            for b in range(NB):
                n = counts[b]
                o = offs[b]
                # prefetch next block's x on the Scalar ring
                if b + 1 < NB:
                    xt_next = sb.tile(
                        [P, KH, CMAX], F16, tag="x", bufs=2, name=f"x{b + 1}"
                    )
                    issue_x(b + 1, xt_next)
                else:
                    xt_next = None

                hb = sb.tile([P, KD, CMAX], F16, tag="h", bufs=2, name=f"h{b}")
                yb = sb.tile([P, MH, CMAX], F16, tag="y", bufs=2, name=f"y{b}")

                # ---- Phase 1: gate/up -> h, one i-tile (128 rows) at a time
                for m in range(MI):
                    wgt = sb.tile([P, KH * P], F16, tag="wg", bufs=6, name=f"wg{b}_{m}")
                    nc.sync.dma_start(out=wgt, in_=Wg_a[b, m])
                    wut = sb.tile([P, KH * P], F16, tag="wu", bufs=6, name=f"wu{b}_{m}")
                    nc.sync.dma_start(out=wut, in_=Wu_a[b, m])
                    if b == 0 and m == 0:
                        # tail of block-0 x after wg0/wu0 on the Sync ring
                        # (all x emitted before any matmul that reads it —
                        # dep tracking is program-order)
                        nc.sync.dma_start(
                            out=xt[:, 8:12, :n], in_=xT_r[:, 8:12, o : o + n]
                        )
                        nc.sync.dma_start(
                            out=xt[:, 12:KH, :n], in_=xT_r[:, 12:KH, o : o + n]
                        )
                    wg_r = wgt.rearrange("p (k i) -> p k i", i=P)
                    wu_r = wut.rearrange("p (k i) -> p k i", i=P)

                    pg = ps.tile([P, n], F32, tag="g", bufs=2, name=f"pg{b}_{m}")
                    pu = ps.tile([P, n], F32, tag="u", bufs=2, name=f"pu{b}_{m}")
                    if b == 0 and m == 0:
                        # startup: interleave gate/up per k-chunk so x is
                        # consumed at half the rate while it streams in —
                        # keeps the PE busy right after warmup (no HAM
                        # re-throttle) with only wg0+wu0+chunk0 resident
                        for k in range(KH):
                            nc.tensor.matmul(
                                pg,
                                wg_r[:, k, :],
                                xt[:, k, :n],
                                start=(k == 0),
                                stop=(k == KH - 1),
                            )
                            nc.tensor.matmul(
                                pu,
                                wu_r[:, k, :],
                                xt[:, k, :n],
                                start=(k == 0),
                                stop=(k == KH - 1),
                            )
                    else:
                        for k in range(KH):
                            nc.tensor.matmul(
                                pg,
                                wg_r[:, k, :],
                                xt[:, k, :n],
                                start=(k == 0),
                                stop=(k == KH - 1),
                            )
                        for k in range(KH):
                            nc.tensor.matmul(
                                pu,
                                wu_r[:, k, :],
                                xt[:, k, :n],
                                start=(k == 0),
                                stop=(k == KH - 1),
                            )
                    tg = sb.tile([P, n], F32, tag="tg", bufs=2, name=f"tg{b}_{m}")
                    nc.scalar.activation(tg, pg, gelu)
                    nc.vector.tensor_mul(hb[:, m, :n], tg, pu)

                # down-proj weights for this block (2 halves, Sync ring)
                wdt = sb.tile([P, MH, KD, P], F16, tag="wd", bufs=2, name=f"wd{b}")
                half = (MH // 2) * KD * P
                wdt_f = wdt.rearrange("p m k i -> p (m k i)")
                nc.sync.dma_start(out=wdt_f[:, :half], in_=Wd_a[b, :, :half])
                nc.sync.dma_start(out=wdt_f[:, half:], in_=Wd_a[b, :, half:])

                # ---- Phase 2: down -> y partial, one H-tile at a time
                last_block = b == NB - 1
                for mh in range(MH):
                    if last_block and mh == MH - 1:
                        # final tile: two half-width groups so the first
                        # half's copy+DMA hides under the second's matmuls
                        nh = ((n // 2) + 3) // 4 * 4
                        for c0, c1 in ((0, nh), (nh, n)):
                            pd = ps.tile(
                                [P, c1 - c0], F32, tag="d", bufs=3, name=f"pd{b}_{mh}_{c0}"
                            )
                            for k in range(KD):
                                nc.tensor.matmul(
                                    pd,
                                    wdt[:, mh, k, :],
                                    hb[:, k, c0:c1],
                                    start=(k == 0),
                                    stop=(k == KD - 1),
                                )
                            nc.vector.tensor_copy(yb[:, mh, c0:c1], pd)
                            nc.scalar.dma_start(
                                out=yT_r[:, mh : mh + 1, o + c0 : o + c1],
                                in_=yb[:, mh : mh + 1, c0:c1],
                            )
                        continue
                    pd = ps.tile([P, n], F32, tag="d", bufs=3, name=f"pd{b}_{mh}")
                    for k in range(KD):
                        nc.tensor.matmul(
                            pd,
                            wdt[:, mh, k, :],
                            hb[:, k, :n],
                            start=(k == 0),
                            stop=(k == KD - 1),
                        )
                    nc.vector.tensor_copy(yb[:, mh, :n], pd)
                    # y partials drain on the otherwise-idle gpsimd SWDGE
                    # ring so they never congest the scalar queue (which
                    # must stay responsive for gelu ACTIVATEs) nor delay
                    # the weight stream on sync
                    if last_block and mh == 13:
                        # tail tiles ride the idle HWDGE rings (sync here,
                        # scalar below) so they overlap the final matmuls;
                        # a gpsimd DMA this late would leave a multi-us
                        # SWDGE drain in the teardown
                        nc.sync.dma_start(
                            out=yT_r[:, 12 : mh + 1, o : o + n],
                            in_=yb[:, 12 : mh + 1, :n],
                        )
                    elif last_block and mh == 14:
                        nc.scalar.dma_start(
                            out=yT_r[:, mh : mh + 1, o : o + n],
                            in_=yb[:, mh : mh + 1, :n],
                        )
                    elif mh % 4 == 3 and not (last_block and mh == MH - 1):
                        nc.gpsimd.dma_start(
                            out=yT_r[:, mh - 3 : mh + 1, o : o + n],
                            in_=yb[:, mh - 3 : mh + 1, :n],
                        )
                xt = xt_next

    nc.compile()
    return nc


def _get_program(counts: tuple[int, ...]) -> "bass.Bass":
    if counts not in _PROGRAM_CACHE:
        _PROGRAM_CACHE[counts] = _build_program(counts)
    return _PROGRAM_CACHE[counts]


def _prep_w_gu_slice(w16, r):
    """[H, I] fp16 -> [MI, P, KH*P] for I-slice r (per-tile contiguous)."""
    s = w16[:, r * SI : (r + 1) * SI]  # [2048, 512]
    return np.ascontiguousarray(
        s.reshape(KH, P, MI, P).transpose(2, 1, 0, 3)
    ).reshape(MI, P, KH * P)


def _prep_w_d_slice(w16, r):
    """[I, H] fp16 -> [P, MH*KD*P] for I-slice r: [p_i, (mh, k, p_h)]."""
    s = w16[r * SI : (r + 1) * SI, :]  # [512, 2048]
    return np.ascontiguousarray(
        s.reshape(KD, P, MH, P).transpose(1, 2, 0, 3)
    ).reshape(P, MH * KD * P)


def kernel(x, selected_experts, routing_weights, Wg, Wu, Wd):
    global LAST_RESULTS
    x = np.asarray(x, dtype=np.float32)
    se = np.asarray(selected_experts).astype(np.int64)
    rw = np.asarray(routing_weights).astype(np.float32)
    Wg = np.asarray(Wg, dtype=np.float32)
    Wu = np.asarray(Wu, dtype=np.float32)
    Wd = np.asarray(Wd, dtype=np.float32)

    T, K = se.shape
    assert x.shape == (T, H) and Wg.shape == (E, H, I) and Wd.shape == (E, I, H)

    # Dense route matrix, identical to the reference's scatter-add (merges
    # duplicate expert picks within a token by summing their weights).
    flat_t = np.repeat(np.arange(T), K)
    flat_e = se.ravel()
    route = np.zeros((T, E), np.float32)
    np.add.at(route, (flat_t, flat_e), rw.ravel())
    present = np.zeros((T, E), bool)
    present[flat_t, flat_e] = True

    # Blocks: one per expert (split if > CMAX), descending size so the
    # device tail lands on the smallest block.
    blocks = []  # (expert, token_index_array)
    for e in range(E):
        ix = np.nonzero(present[:, e])[0]
        for s in range(0, len(ix), CMAX):
            chunk = ix[s : s + CMAX]
            if len(chunk):
                blocks.append((e, chunk))
    blocks.sort(key=lambda be: -len(be[1]))
    if len(blocks) > 1:
        # smallest block first (lightest startup x transfer), rest descending
        blocks = blocks[-1:] + blocks[:-1]
    counts = tuple(max(8, -(-len(ix) // 4) * 4) for _, ix in blocks)
    offs = np.concatenate([[0], np.cumsum(counts)])
    A = int(offs[-1])

    nc = _get_program(counts)

    xT_all = np.zeros((H, A), np.float16)
    for b, (_, ix) in enumerate(blocks):
        xT_all[:, offs[b] : offs[b] + len(ix)] = x[ix].T.astype(np.float16)

    Wg16 = Wg.astype(np.float16)
    Wu16 = Wu.astype(np.float16)
    Wd16 = Wd.astype(np.float16)

    in_maps = []
    for r in range(NCORES):
        wg_r = np.stack([_prep_w_gu_slice(Wg16[e], r) for e, _ in blocks])
        wu_r = np.stack([_prep_w_gu_slice(Wu16[e], r) for e, _ in blocks])
        wd_r = np.stack([_prep_w_d_slice(Wd16[e], r) for e, _ in blocks])
        in_maps.append({"xT": xT_all, "Wg": wg_r, "Wu": wu_r, "Wd": wd_r})

    res = run_bass_kernel_spmd(nc, in_maps, core_ids=list(range(NCORES)))
    LAST_RESULTS = res

    ysum = np.zeros((H, A), np.float32)
    for r in range(NCORES):
        ysum += res.results[r]["yT"].astype(np.float32)

    out = np.zeros((T, H), np.float32)
    for b, (e, ix) in enumerate(blocks):
        seg = ysum[:, offs[b] : offs[b] + len(ix)]
        out[ix] += route[ix, e][:, None] * seg.T
    return out


# revision 22
# speedup vs baseline: 1.0106x; 1.0000x over previous
"""MoE (Gemma-style 8-expert top-2) Trainium2 kernel, v2.

Strategy (intermediate-dim sliced expert parallelism over 8 NeuronCores):
  - Every core processes ALL token->expert assignments, but only a 1/8
    slice (512 cols) of the intermediate dimension I=4096 of every expert.
    Per-core work is therefore sum(C_e)/8 token-equivalents regardless of
    how unbalanced the routing is (vs max(C_e) for one-expert-per-core),
    while per-core weight traffic is identical (1/8 of every expert's
    Wg/Wu/Wd = one full expert's worth).
  - Host: dedup (token,expert) pairs, gather+transpose x into a single
    xT buffer [H, A] (per-expert segments, zero-padded to multiples of 8),
    prepack per-core weight slices so every device DMA is 2D-contiguous.
  - Device (per core r), for each expert block b (descending token count):
        gateT[i,c] = sum_h Wg[h, r*512+i] * xT[h,c]    i in [0,512)
        upT   likewise                                  (fp16 in, fp32 acc)
        hT    = gelu_tanh(gateT) * upT                  [512, C_b] fp16
        ypartT[h,c] = sum_i Wd[r*512+i, h] * hT[i,c]    [H, C_b] -> fp16 out
    Weights stream on the Sync HWDGE ring; x in / y out stream on the
    Scalar (Activation) HWDGE ring so big x transfers never head-of-line
    block the weight stream.  A short 5-matmul warmup trips the PE HAM
    clock-gate while the first DMAs land.
  - Host: sum the 8 cores' fp16 partial yT (f32 accumulate), then
    combine: out[t] += route[t,e] * ysum[:, pos].T, route identical to
    the reference's scatter-add.
"""

import numpy as np

import concourse.bass as bass
import concourse.mybir as mybir
import concourse.tile as tile
from concourse import bacc


def _install_ntff_hook_shim():
    """The agent image's `antenv` lacks `axon_hooks`, which bass_utils
    imports unconditionally when tracing under axon.  Provide the module
    and register the ctypes-based NTFF profile hook so BASS_TRACE=1 yields
    real HW profiles.  Degrades silently if anything is missing."""
    import sys
    import types

    try:
        import antenv

        try:
            from antenv import axon_hooks  # noqa: F401

            return
        except ImportError:
            pass
        mod = types.ModuleType("antenv.axon_hooks")
        mod._hook = None
        mod.set_axon_ntff_profile_hook = lambda h: setattr(mod, "_hook", h)
        mod.get_axon_ntff_profile_hook = lambda: mod._hook
        sys.modules["antenv.axon_hooks"] = mod
        antenv.axon_hooks = mod
        import os

        so_path = "/opt/axon/libaxon_pjrt.so"
        if os.path.exists(so_path):
            from trn_agent_boot.trn_boot import _ntff_profile_via_ctypes

            mod._hook = _ntff_profile_via_ctypes(so_path)
    except Exception:
        pass


_install_ntff_hook_shim()

from concourse.bass_utils import run_bass_kernel_spmd

H = 2048
I = 4096
E = 8
NCORES = 8
P = 128
KH = H // P  # 16 contraction chunks for gate/up
SI = I // NCORES  # 512-wide I slice per core
MI = SI // P  # 4 output i-tiles per expert block
KD = SI // P  # 4 contraction chunks for down
MH = H // P  # 16 output tiles of H
CMAX = 504  # max tokens per block (one PSUM fp32 bank)
F32 = mybir.dt.float32
F16 = mybir.dt.float16

# Results of the last device run (for test harnesses to inspect profiling).
LAST_RESULTS = None

_PROGRAM_CACHE: dict[tuple, "bass.Bass"] = {}


def _build_program(counts: tuple[int, ...]) -> "bass.Bass":
    """Bass program for one core: for each block b, expert-MLP over an
    SI-wide I-slice on counts[b] tokens (transposed layout)."""
    NB = len(counts)
    assert all(8 <= n <= CMAX and n % 4 == 0 for n in counts)
    offs = [0]
    for n in counts:
        offs.append(offs[-1] + n)
    A = offs[-1]

    nc = bacc.Bacc("TRN2", target_bir_lowering=False)

    xT = nc.dram_tensor("xT", [H, A], F16, kind="ExternalInput")
    Wg = nc.dram_tensor("Wg", [NB, MI, P, KH * P], F16, kind="ExternalInput")
    Wu = nc.dram_tensor("Wu", [NB, MI, P, KH * P], F16, kind="ExternalInput")
    Wd = nc.dram_tensor("Wd", [NB, P, MH * KD * P], F16, kind="ExternalInput")
    yT = nc.dram_tensor("yT", [H, A], F16, kind="ExternalOutput")

    xT_r = xT.rearrange("(k p) a -> p k a", p=P)  # [128, 16, A]
    yT_r = yT.rearrange("(m p) a -> p m a", p=P)  # [128, 16, A]
    Wg_a, Wu_a, Wd_a = Wg.ap(), Wu.ap(), Wd.ap()

    gelu = mybir.ActivationFunctionType.Gelu_apprx_tanh

    with tile.TileContext(nc) as tc:
        with (
            tc.tile_pool(name="sb", bufs=1) as sb,
            tc.tile_pool(name="ps", bufs=1, space="PSUM") as ps,
        ):
            # --- PE warmup: a few dummy matmuls trip the HAM clock-gate to
            # 2.4 GHz while the first DMAs land.
            wz = sb.tile([P, P], F16, tag="wz", bufs=1)
            xz = sb.tile([P, CMAX], F16, tag="xz", bufs=1)
            nc.vector.memset(wz, 0.0)
            nc.vector.memset(xz, 0.0)
            pw = ps.tile([P, CMAX], F32, tag="pw", bufs=1)
            # two-stage warmup: big matmuls trip the HAM activity window,
            # then tiny ones keep PE busy through the dep-arrival jitter
            # with minimal drain latency once real work is ready
            for _ in range(12):
                nc.tensor.matmul(pw, wz, xz, start=True, stop=True)
            for _ in range(30):
                nc.tensor.matmul(pw[:, :128], wz, xz[:, :128], start=True, stop=True)

            def issue_x(b, xtile):
                n = counts[b]
                o = offs[b]
                if b == 0:
                    # block 0's first x chunks ride the Scalar ring in
                    # consumption-order pieces; the tail (8:16) rides the
                    # Sync ring after wg0/wu0 (emitted in the gate loop).
                    for s0, s1 in [(0, 3), (3, 8)]:
                        nc.scalar.dma_start(
                            out=xtile[:, s0:s1, :n], in_=xT_r[:, s0:s1, o : o + n]
                        )
                else:
                    nc.scalar.dma_start(
                        out=xtile[:, 0:8, :n], in_=xT_r[:, 0:8, o : o + n]
                    )
                    nc.scalar.dma_start(
                        out=xtile[:, 8:KH, :n], in_=xT_r[:, 8:KH, o : o + n]
                    )

            xt = sb.tile([P, KH, CMAX], F16, tag="x", bufs=2, name="x0")
            issue_x(0, xt)

            for b in range(NB):
                n = counts[b]
                o = offs[b]
                # prefetch next block's x on the Scalar ring
                if b + 1 < NB:
                    xt_next = sb.tile(
                        [P, KH, CMAX], F16, tag="x", bufs=2, name=f"x{b + 1}"
                    )
                    issue_x(b + 1, xt_next)
                else:
                    xt_next = None

                hb = sb.tile([P, KD, CMAX], F16, tag="h", bufs=2, name=f"h{b}")
                yb = sb.tile([P, MH, CMAX], F16, tag="y", bufs=2, name=f"y{b}")

                # ---- Phase 1: gate/up -> h, one i-tile (128 rows) at a time
                for m in range(MI):
                    wgt = sb.tile([P, KH * P], F16, tag="wg", bufs=6, name=f"wg{b}_{m}")
                    nc.sync.dma_start(out=wgt, in_=Wg_a[b, m])
                    wut = sb.tile([P, KH * P], F16, tag="wu", bufs=6, name=f"wu{b}_{m}")
                    nc.sync.dma_start(out=wut, in_=Wu_a[b, m])
                    if b == 0 and m == 0:
                        # tail of block-0 x after wg0/wu0 on the Sync ring
                        # (all x emitted before any matmul that reads it —
                        # dep tracking is program-order)
                        nc.sync.dma_start(
                            out=xt[:, 8:12, :n], in_=xT_r[:, 8:12, o : o + n]
                        )
                        nc.sync.dma_start(
                            out=xt[:, 12:KH, :n], in_=xT_r[:, 12:KH, o : o + n]
                        )
                    wg_r = wgt.rearrange("p (k i) -> p k i", i=P)
                    wu_r = wut.rearrange("p (k i) -> p k i", i=P)

                    pg = ps.tile([P, n], F32, tag="g", bufs=2, name=f"pg{b}_{m}")
                    pu = ps.tile([P, n], F32, tag="u", bufs=2, name=f"pu{b}_{m}")
                    if b == 0 and m == 0:
                        # startup: interleave gate/up per k-chunk so x is
                        # consumed at half the rate while it streams in —
                        # keeps the PE busy right after warmup (no HAM
                        # re-throttle) with only wg0+wu0+chunk0 resident
                        for k in range(KH):
                            nc.tensor.matmul(
                                pg,
                                wg_r[:, k, :],
                                xt[:, k, :n],
                                start=(k == 0),
                                stop=(k == KH - 1),
                            )
                            nc.tensor.matmul(
                                pu,
                                wu_r[:, k, :],
                                xt[:, k, :n],
                                start=(k == 0),
                                stop=(k == KH - 1),
                            )
                    else:
                        for k in range(KH):
                            nc.tensor.matmul(
                                pg,
                                wg_r[:, k, :],
                                xt[:, k, :n],
                                start=(k == 0),
                                stop=(k == KH - 1),
                            )
                        for k in range(KH):
                            nc.tensor.matmul(
                                pu,
                                wu_r[:, k, :],
                                xt[:, k, :n],
                                start=(k == 0),
                                stop=(k == KH - 1),
                            )
                    tg = sb.tile([P, n], F32, tag="tg", bufs=2, name=f"tg{b}_{m}")
                    nc.scalar.activation(tg, pg, gelu)
                    nc.vector.tensor_mul(hb[:, m, :n], tg, pu)

                # down-proj weights for this block (2 halves, Sync ring)
                wdt = sb.tile([P, MH, KD, P], F16, tag="wd", bufs=2, name=f"wd{b}")
                half = (MH // 2) * KD * P
                wdt_f = wdt.rearrange("p m k i -> p (m k i)")
                nc.sync.dma_start(out=wdt_f[:, :half], in_=Wd_a[b, :, :half])
                nc.sync.dma_start(out=wdt_f[:, half:], in_=Wd_a[b, :, half:])

                # ---- Phase 2: down -> y partial, one H-tile at a time
                last_block = b == NB - 1
                for mh in range(MH):
                    if last_block and mh == MH - 1:
                        # final tile: two half-width groups so the first
                        # half's copy+DMA hides under the second's matmuls
                        nh = ((n // 2) + 3) // 4 * 4
                        for c0, c1 in ((0, nh), (nh, n)):
                            pd = ps.tile(
                                [P, c1 - c0], F32, tag="d", bufs=3, name=f"pd{b}_{mh}_{c0}"
                            )
                            for k in range(KD):
                                nc.tensor.matmul(
                                    pd,
                                    wdt[:, mh, k, :],
                                    hb[:, k, c0:c1],
                                    start=(k == 0),
                                    stop=(k == KD - 1),
                                )
                            nc.vector.tensor_copy(yb[:, mh, c0:c1], pd)
                            nc.scalar.dma_start(
                                out=yT_r[:, mh : mh + 1, o + c0 : o + c1],
                                in_=yb[:, mh : mh + 1, c0:c1],
                            )
                        continue
                    pd = ps.tile([P, n], F32, tag="d", bufs=3, name=f"pd{b}_{mh}")
                    for k in range(KD):
                        nc.tensor.matmul(
                            pd,
                            wdt[:, mh, k, :],
                            hb[:, k, :n],
                            start=(k == 0),
                            stop=(k == KD - 1),
                        )
                    nc.vector.tensor_copy(yb[:, mh, :n], pd)
                    # y partials drain on the otherwise-idle gpsimd SWDGE
                    # ring so they never congest the scalar queue (which
                    # must stay responsive for gelu ACTIVATEs) nor delay
                    # the weight stream on sync
                    if last_block and mh == 13:
                        # tail tiles ride the idle HWDGE rings (sync here,
                        # scalar below) so they overlap the final matmuls;
                        # a gpsimd DMA this late would leave a multi-us
                        # SWDGE drain in the teardown
                        nc.sync.dma_start(
                            out=yT_r[:, 12 : mh + 1, o : o + n],
                            in_=yb[:, 12 : mh + 1, :n],
                        )
                    elif last_block and mh == 14:
                        nc.scalar.dma_start(
                            out=yT_r[:, mh : mh + 1, o : o + n],
                            in_=yb[:, mh : mh + 1, :n],
                        )
                    elif mh % 4 == 3 and not (last_block and mh == MH - 1):
                        nc.gpsimd.dma_start(
                            out=yT_r[:, mh - 3 : mh + 1, o : o + n],
                            in_=yb[:, mh - 3 : mh + 1, :n],
                        )
                xt = xt_next

    nc.compile()
    return nc


def _get_program(counts: tuple[int, ...]) -> "bass.Bass":
    if counts not in _PROGRAM_CACHE:
        _PROGRAM_CACHE[counts] = _build_program(counts)
    return _PROGRAM_CACHE[counts]


def _prep_w_gu_slice(w16, r):
    """[H, I] fp16 -> [MI, P, KH*P] for I-slice r (per-tile contiguous)."""
    s = w16[:, r * SI : (r + 1) * SI]  # [2048, 512]
    return np.ascontiguousarray(
        s.reshape(KH, P, MI, P).transpose(2, 1, 0, 3)
    ).reshape(MI, P, KH * P)


def _prep_w_d_slice(w16, r):
    """[I, H] fp16 -> [P, MH*KD*P] for I-slice r: [p_i, (mh, k, p_h)]."""
    s = w16[r * SI : (r + 1) * SI, :]  # [512, 2048]
    return np.ascontiguousarray(
        s.reshape(KD, P, MH, P).transpose(1, 2, 0, 3)
    ).reshape(P, MH * KD * P)


def kernel(x, selected_experts, routing_weights, Wg, Wu, Wd):
    global LAST_RESULTS
    x = np.asarray(x, dtype=np.float32)
    se = np.asarray(selected_experts).astype(np.int64)
    rw = np.asarray(routing_weights).astype(np.float32)
    Wg = np.asarray(Wg, dtype=np.float32)
    Wu = np.asarray(Wu, dtype=np.float32)
    Wd = np.asarray(Wd, dtype=np.float32)

    T, K = se.shape
    assert x.shape == (T, H) and Wg.shape == (E, H, I) and Wd.shape == (E, I, H)

    # Dense route matrix, identical to the reference's scatter-add (merges
    # duplicate expert picks within a token by summing their weights).
    flat_t = np.repeat(np.arange(T), K)
    flat_e = se.ravel()
    route = np.zeros((T, E), np.float32)
    np.add.at(route, (flat_t, flat_e), rw.ravel())
    present = np.zeros((T, E), bool)
    present[flat_t, flat_e] = True

    # Blocks: one per expert (split if > CMAX), descending size so the
    # device tail lands on the smallest block.
    blocks = []  # (expert, token_index_array)
    for e in range(E):
        ix = np.nonzero(present[:, e])[0]
        for s in range(0, len(ix), CMAX):
            chunk = ix[s : s + CMAX]
            if len(chunk):
                blocks.append((e, chunk))
    blocks.sort(key=lambda be: -len(be[1]))
    if len(blocks) > 1:
        # smallest block first (lightest startup x transfer), rest descending
        blocks = blocks[-1:] + blocks[:-1]
    counts = tuple(max(8, -(-len(ix) // 4) * 4) for _, ix in blocks)
    offs = np.concatenate([[0], np.cumsum(counts)])
    A = int(offs[-1])

    nc = _get_program(counts)

    xT_all = np.zeros((H, A), np.float16)
    for b, (_, ix) in enumerate(blocks):
        xT_all[:, offs[b] : offs[b] + len(ix)] = x[ix].T.astype(np.float16)

    Wg16 = Wg.astype(np.float16)
    Wu16 = Wu.astype(np.float16)
    Wd16 = Wd.astype(np.float16)

    in_maps = []
    for r in range(NCORES):
        wg_r = np.stack([_prep_w_gu_slice(Wg16[e], r) for e, _ in blocks])
        wu_r = np.stack([_prep_w_gu_slice(Wu16[e], r) for e, _ in blocks])
        wd_r = np.stack([_prep_w_d_slice(Wd16[e], r) for e, _ in blocks])
        in_maps.append({"xT": xT_all, "Wg": wg_r, "Wu": wu_r, "Wd": wd_r})

    res = run_bass_kernel_spmd(nc, in_maps, core_ids=list(range(NCORES)))
    LAST_RESULTS = res

    ysum = np.zeros((H, A), np.float32)
    for r in range(NCORES):
        ysum += res.results[r]["yT"].astype(np.float32)

    out = np.zeros((T, H), np.float32)
    for b, (e, ix) in enumerate(blocks):
        seg = ysum[:, offs[b] : offs[b] + len(ix)]
        out[ix] += route[ix, e][:, None] * seg.T
    return out
